# revision 4
# baseline (speedup 1.0000x reference)
# Multi-head self-attention with RoPE on 8 Trainium2 NeuronCores.
#
# Sharding: batch x head-group. Core c handles batch b = c//4 and heads
# 4*(c%4) .. 4*(c%4)+3 (4 of 16 heads). Each core computes Q/K/V
# projections for its heads from the full (transposed) x[b], runs
# attention, and produces a partial output projection
# Y_partial = O_core^T.T @ Wo[rows-of-its-heads]. The host sums the four
# partials per batch and adds the (constant) bias terms.
#
# All matmuls run in float32r (TF32-like, ~1.5e-4 rel rounding) which
# streams at 1 column/cycle on the PE (4x faster than true fp32).
#
# Per-core layouts:
#   XT    [1024 D, 2048 t]  (host-transposed x[b])
#   QT/KT [128 = 2 heads x 64 d(permuted), 2048 t]  x 2 pair-tiles
#   V     16 tiles [128 t-chunk, 4 heads x 65] (65th col = 1.0 -> row sums)
#   S^T   6-bank PSUM ring [128, 3072]; K=64 matmuls for the two heads of a
#         pair run CONCURRENTLY on the PE via row-group tiling (partition
#         bases 0/64), exp'd in groups of 3 banks -> P tiles (f32r)
#   O^T   psum [65, 512]: rows 0..63 = sum_k exp*v, row 64 = Z (denominator)
#   Y     [2048 t, 1024 e] fp32 partial; Y psum reuses the ring banks
#
# RoPE: head-dim rows are pair-interleaved (d' = [0,32,1,33,...]) via a host
# permutation of Wq/Wk columns so the rotate-half partner lives on the
# adjacent partition; a DVE stream_shuffle (pair swap) + 2 muls + 1 add
# apply the rotation with band-replicated, sign-baked cos/sin tables.

import os
import sys

import numpy as np

for _p in ("/opt/trn_rl_repo", os.path.expanduser("~/.axon_site/_ro/trn_rl_repo")):
    if os.path.isdir(_p) and _p not in sys.path:
        sys.path.insert(0, _p)

B, T, D = 2, 2048, 1024
NHEADS, HD, HALF = 16, 64, 32
HPC = 4  # heads per core
N_CORES = 8
ROPE_BASE = 10000.0
SCALE = float(HD) ** -0.5  # 0.125
NDC = D // 128  # 8 contraction chunks for the projections
NKC = T // 128  # 16 k chunks per head
RING_BANKS = 6

_SHUF_MASK = [i ^ 1 for i in range(32)]

_ctx: dict = {}


def _build_nc(iters: int = 0):
    import concourse.bacc as bacc
    import concourse.mybir as mybir
    import concourse.tile as tile

    f32 = mybir.dt.float32
    f32r = mybir.dt.float32r
    u32 = mybir.dt.uint32
    Exp = mybir.ActivationFunctionType.Exp
    MUL = mybir.AluOpType.mult
    ADD = mybir.AluOpType.add

    nc = bacc.Bacc("TRN2", target_bir_lowering=False, debug=False)

    xt_d = nc.dram_tensor("xt", [D, T], f32, kind="ExternalInput").ap()
    wq_d = nc.dram_tensor("wq", [D, 256], f32, kind="ExternalInput").ap()
    wk_d = nc.dram_tensor("wk", [D, 256], f32, kind="ExternalInput").ap()
    wv_d = nc.dram_tensor("wv", [D, 256], f32, kind="ExternalInput").ap()
    wo_d = nc.dram_tensor("wo", [256, D], f32, kind="ExternalInput").ap()
    cos_d = nc.dram_tensor("cosb", [128, T], f32, kind="ExternalInput").ap()
    sin_d = nc.dram_tensor("sinb", [128, T], f32, kind="ExternalInput").ap()
    qb_d = nc.dram_tensor("qb", [128, 2], f32, kind="ExternalInput").ap()
    kb_d = nc.dram_tensor("kb", [128, 2], f32, kind="ExternalInput").ap()
    ones_d = nc.dram_tensor("onesd", [128, 4], f32, kind="ExternalInput").ap()
    y_d = nc.dram_tensor("y", [T, D], f32, kind="ExternalOutput").ap()

    with tile.TileContext(nc) as tc:
        with (
            tc.tile_pool(name="big", bufs=18) as big,
            tc.tile_pool(name="vpool", bufs=16) as vpool,
            tc.tile_pool(name="small", bufs=2) as small,
            tc.tile_pool(name="ypool", bufs=6) as ypool,
            tc.tile_pool(name="pss", bufs=1, space="PSUM") as pss,
            tc.tile_pool(name="psb", bufs=2, space="PSUM") as psb,
        ):

            def body():
                # ---- loads, ordered by first use ----
                def load_w(dram, name):
                    t_ = big.tile([128, 8 * 256], f32r, tag="big", name=name)
                    for ch in range(NDC):
                        nc.sync.dma_start(
                            out=t_[:, ch * 256 : (ch + 1) * 256],
                            in_=dram[ch * 128 : (ch + 1) * 128, :].bitcast(f32r),
                        )
                    return t_

                wq_t = load_w(wq_d, "wq_t")
                xts = []
                for ch in range(NDC):
                    t_ = big.tile([128, T], f32r, tag="big", name=f"xt{ch}")
                    nc.sync.dma_start(
                        out=t_[:], in_=xt_d[ch * 128 : (ch + 1) * 128, :].bitcast(f32r)
                    )
                    xts.append(t_)
                wk_t = load_w(wk_d, "wk_t")
                wv_t = load_w(wv_d, "wv_t")
                qb_t = small.tile([128, 2], f32, tag="bias", name="qb_t")
                nc.scalar.dma_start(out=qb_t[:], in_=qb_d)
                kb_t = small.tile([128, 2], f32, tag="bias", name="kb_t")
                nc.scalar.dma_start(out=kb_t[:], in_=kb_d)
                cos_t = big.tile([128, T], f32r, tag="big", name="cos_t")
                nc.scalar.dma_start(out=cos_t[:], in_=cos_d.bitcast(f32r))
                sin_t = big.tile([128, T], f32r, tag="big", name="sin_t")
                nc.scalar.dma_start(out=sin_t[:], in_=sin_d.bitcast(f32r))

                ring = pss.tile([128, RING_BANKS * 512], f32, tag="ring", name="ring")

                # ---- Q/K projections; eviction = DVE tensor_scalar (+bias) ----
                def project(w_t, bias_t, name, pr):
                    qt = big.tile([128, T], f32r, tag="big", name=f"{name}{pr}")
                    for ts in range(4):
                        ps = psb.tile(
                            [128, 512], f32, tag="b1", name=f"ps_{name}{pr}_{ts}"
                        )
                        for ch in range(NDC):
                            c0 = ch * 256 + pr * 128
                            nc.tensor.matmul(
                                ps[:],
                                w_t[:, c0 : c0 + 128],
                                xts[ch][:, ts * 512 : (ts + 1) * 512],
                                start=(ch == 0),
                                stop=(ch == NDC - 1),
                            )
                        nc.vector.tensor_scalar_add(
                            qt[:, ts * 512 : (ts + 1) * 512],
                            ps[:],
                            bias_t[:, pr : pr + 1],
                        )
                    return qt

                def rope(t_, name):
                    sh = big.tile([128, T], f32r, tag="big", name=f"sh_{name}")
                    for s0 in range(0, T, 1024):
                        sl = slice(s0, s0 + 1024)
                        nc.vector.stream_shuffle(
                            sh.bitcast(u32)[:, sl], t_.bitcast(u32)[:, sl], _SHUF_MASK
                        )
                        nc.vector.tensor_tensor(
                            out=t_[:, sl], in0=t_[:, sl], in1=cos_t[:, sl], op=MUL
                        )
                        nc.vector.tensor_tensor(
                            out=sh[:, sl], in0=sh[:, sl], in1=sin_t[:, sl], op=MUL
                        )
                        nc.vector.tensor_tensor(
                            out=t_[:, sl], in0=t_[:, sl], in1=sh[:, sl], op=ADD
                        )

                qts, kts = [], []
                for pr in range(2):
                    qts.append(project(wq_t, qb_t, "qt", pr))
                    kts.append(project(wk_t, kb_t, "kt", pr))
                    rope(qts[pr], f"q{pr}")
                    rope(kts[pr], f"k{pr}")

                # ---- V projection ----
                vts = []
                for tk in range(NKC):
                    vt = vpool.tile([128, HPC * 65], f32r, tag="v", name=f"v{tk}")
                    nc.scalar.dma_start(
                        out=vt.rearrange("p (h c) -> p h c", c=65)[:, :, 64:65],
                        in_=ones_d.rearrange("p (h c) -> p h c", c=1).bitcast(f32r),
                    )
                    ps = psb.tile([128, 256], f32, tag="b1", name=f"psv{tk}")
                    for ch in range(NDC):
                        nc.tensor.matmul(
                            ps[:],
                            xts[ch][:, tk * 128 : (tk + 1) * 128],
                            wv_t[:, ch * 256 : (ch + 1) * 256],
                            start=(ch == 0),
                            stop=(ch == NDC - 1),
                        )
                    nc.vector.tensor_copy(
                        vt.rearrange("p (h c) -> p h c", c=65)[:, :, 0:64],
                        ps.rearrange("p (h c) -> p h c", c=64),
                    )
                    vts.append(vt)

                # ---- attention: per head-pair, both heads concurrent on PE ----
                ot0 = big.tile([128, T], f32r, tag="big", name="ot0")
                ot1 = big.tile([128, T], f32r, tag="big", name="ot1")
                ots = [ot0, ot1]
                ROUNDS = [(0, 1, 2), (3, 4, 5), (6, 7, 8), (9, 10, 11), (12, 13, 14), (15,)]
                for pr in range(2):
                    for qi in range(4):
                        qs = qi * 512
                        oaccs = []
                        for hh in range(2):
                            oaccs.append(
                                psb.tile([65, 512], f32, tag="b1", name=f"o_{pr}_{qi}_{hh}")
                            )
                        for ri, rnd in enumerate(ROUNDS):
                            # fill banks: chunk kc head hh -> bank 2*i+hh
                            for i, kc in enumerate(rnd):
                                for hh in range(2):
                                    nc.tensor.matmul(
                                        ring[:, (2 * i + hh) * 512 : (2 * i + hh + 1) * 512],
                                        kts[pr][hh * 64 : hh * 64 + 64, kc * 128 : (kc + 1) * 128],
                                        qts[pr][hh * 64 : hh * 64 + 64, qs : qs + 512],
                                        start=True,
                                        stop=True,
                                    )
                            nb = 2 * len(rnd)
                            for g0 in range(0, nb, 3):
                                gw = min(3, nb - g0)
                                pt = big.tile(
                                    [128, 512 * gw],
                                    f32r,
                                    tag="big",
                                    name=f"p_{pr}_{qi}_{ri}_{g0}",
                                )
                                nc.scalar.activation(
                                    pt[:],
                                    ring[:, g0 * 512 : (g0 + gw) * 512],
                                    Exp,
                                    bias=0.0,
                                    scale=SCALE,
                                )
                                for j in range(gw):
                                    i, hh = divmod(g0 + j, 2)
                                    kc = rnd[i]
                                    nc.tensor.matmul(
                                        oaccs[hh][:],
                                        vts[kc][:, (2 * pr + hh) * 65 : (2 * pr + hh + 1) * 65],
                                        pt[:, j * 512 : (j + 1) * 512],
                                        start=(kc == 0),
                                        stop=(kc == NKC - 1),
                                    )
                        for hh in range(2):
                            rt = small.tile([1, 512], f32, tag="rt", name=f"rt_{pr}_{qi}_{hh}")
                            nc.vector.reciprocal(rt[0:1, :], oaccs[hh][64:65, :])
                            rb = small.tile([64, 512], f32, tag="rb", name=f"rb_{pr}_{qi}_{hh}")
                            nc.gpsimd.partition_broadcast(rb[:, :], rt[0:1, :], channels=64)
                            nc.vector.tensor_tensor(
                                out=ots[pr][hh * 64 : hh * 64 + 64, qs : qs + 512],
                                in0=oaccs[hh][0:64, :],
                                in1=rb[:, :],
                                op=MUL,
                            )

                # ---- output projection partial (Y psum reuses ring banks) ----
                wo_t = big.tile([128, 2048], f32r, tag="big", name="wo_t")
                for r in range(2):
                    nc.sync.dma_start(
                        out=wo_t[:, r * 1024 : (r + 1) * 1024],
                        in_=wo_d[r * 128 : (r + 1) * 128, :].bitcast(f32r),
                    )
                for i, (tt, eh) in enumerate(
                    [(tt, eh) for tt in range(16) for eh in range(2)]
                ):
                    bk = i % RING_BANKS
                    yps = ring[:, bk * 512 : (bk + 1) * 512]
                    for r in range(2):
                        nc.tensor.matmul(
                            yps,
                            ots[r][:, tt * 128 : (tt + 1) * 128],
                            wo_t[:, r * 1024 + eh * 512 : r * 1024 + (eh + 1) * 512],
                            start=(r == 0),
                            stop=(r == 1),
                        )
                    ysb = ypool.tile([128, 512], f32, tag="y", name=f"y_{tt}_{eh}")
                    nc.vector.tensor_copy(ysb[:], yps)
                    nc.sync.dma_start(
                        out=y_d[tt * 128 : (tt + 1) * 128, eh * 512 : (eh + 1) * 512],
                        in_=ysb[:],
                    )

            if iters:
                with tc.For_i(0, iters, 1) as _iv:
                    body()
            else:
                body()

    nc.compile()
    return nc


def _host_inputs(x, wq_w, wq_b, wk_w, wk_b, wv_w, wv_b, wo_w, wo_b):
    """Build the 8 per-core input maps (all host-side slicing/transposes)."""
    f = np.float32
    x = np.asarray(x, f)
    wq_w = np.asarray(wq_w, f)
    wk_w = np.asarray(wk_w, f)
    wv_w = np.asarray(wv_w, f)
    wo_w = np.asarray(wo_w, f)
    wq_b = np.asarray(wq_b, f)
    wk_b = np.asarray(wk_b, f)
    wv_b = np.asarray(wv_b, f)
    wo_b = np.asarray(wo_b, f)

    # RoPE tables in fp32, mirroring the reference formulas.
    pos = np.arange(T, dtype=f)[:, None]
    idx = np.arange(HALF, dtype=f)[None, :]
    inv_freq = (f(1.0) / (f(ROPE_BASE) ** (idx / f(HALF)))).astype(f)
    ang = pos * inv_freq  # [T, 32]
    cosv, sinv = np.cos(ang).astype(f), np.sin(ang).astype(f)
    cos64 = np.repeat(cosv.T, 2, axis=0)  # [64, T]
    sin64 = np.repeat(sinv.T, 2, axis=0)
    sin64[0::2] *= -1  # rows 2j: -sin, rows 2j+1: +sin
    cos128 = np.ascontiguousarray(np.tile(cos64, (2, 1)))
    sin128 = np.ascontiguousarray(np.tile(sin64, (2, 1)))

    perm64 = np.empty(64, np.int64)
    perm64[0::2] = np.arange(32)
    perm64[1::2] = np.arange(32) + 32

    xts = [np.ascontiguousarray(x[b].T) for b in range(B)]
    ones_pad = np.ones((128, HPC), f)

    in_maps = []
    for c in range(N_CORES):
        b, g = c // 4, c % 4
        heads = np.arange(4 * g, 4 * g + 4)
        qk_cols = np.concatenate([h * 64 + perm64 for h in heads])
        v_cols = np.concatenate([np.arange(h * 64, (h + 1) * 64) for h in heads])
        in_maps.append(
            {
                "xt": xts[b],
                "wq": np.ascontiguousarray(wq_w[:, qk_cols]),
                "wk": np.ascontiguousarray(wk_w[:, qk_cols]),
                "wv": np.ascontiguousarray(wv_w[:, v_cols]),
                "wo": np.ascontiguousarray(wo_w[v_cols, :]),
                "cosb": cos128,
                "sinb": sin128,
                "qb": np.ascontiguousarray(wq_b[qk_cols].reshape(2, 128).T),
                "kb": np.ascontiguousarray(wk_b[qk_cols].reshape(2, 128).T),
                "onesd": ones_pad,
            }
        )

    beff = (wo_b.astype(np.float64) + wv_b.astype(np.float64) @ wo_w.astype(np.float64)).astype(f)
    return in_maps, beff


def kernel(x, wq_w, wq_b, wk_w, wk_b, wv_w, wv_b, wo_w, wo_b):
    from concourse import bass2jax

    in_maps, beff = _host_inputs(
        x, wq_w, wq_b, wk_w, wk_b, wv_w, wv_b, wo_w, wo_b
    )
    if "nc" not in _ctx:
        _ctx["nc"] = _build_nc(0)
    res = bass2jax.run_bass_via_pjrt(_ctx["nc"], in_maps, n_cores=N_CORES)
    y = np.empty((B, T, D), np.float32)
    for b in range(B):
        acc = res[4 * b]["y"].copy()
        for g in range(1, 4):
            acc += res[4 * b + g]["y"]
        y[b] = acc + beff[None, :]
    return y


# revision 7
# speedup vs baseline: 1.2945x; 1.2945x over previous
# Multi-head self-attention with RoPE on 8 Trainium2 NeuronCores.
#
# Sharding: batch x head-group. Core c handles batch b = c//4 and heads
# 4*(c%4) .. 4*(c%4)+3 (4 of 16 heads). Each core computes Q/K/V
# projections for its heads from the full (transposed) x[b], runs
# attention, and produces a partial output projection
# Y_partial = O_core^T.T @ Wo[rows-of-its-heads]. The host sums the four
# partials per batch and adds the (constant) bias terms.
#
# All matmuls run in float32r (TF32-like, ~1.5e-4 rel rounding) which
# streams at 1 column/cycle on the PE (4x faster than true fp32).
#
# Per-core layouts:
#   XT    [1024 D, 2048 t]  (host-transposed x[b])
#   QT/KT [128 = 2 heads x 64 d(permuted), 2048 t]  x 2 pair-tiles
#   V     16 tiles [128 t-chunk, 4 heads x 65] (65th col = 1.0 -> row sums)
#   S^T   6-bank PSUM ring [128, 3072]; K=64 matmuls for the two heads of a
#         pair run CONCURRENTLY on the PE via row-group tiling (partition
#         bases 0/64), exp'd in groups of 3 banks -> P tiles (f32r)
#   O^T   psum [65, 512]: rows 0..63 = sum_k exp*v, row 64 = Z (denominator)
#   Y     [2048 t, 1024 e] fp32 partial; Y psum reuses the ring banks
#
# RoPE: head-dim rows are pair-interleaved (d' = [0,32,1,33,...]) via a host
# permutation of Wq/Wk columns so the rotate-half partner lives on the
# adjacent partition; a DVE stream_shuffle (pair swap) + 2 muls + 1 add
# apply the rotation with band-replicated, sign-baked cos/sin tables.

import os
import sys

import numpy as np

try:
    import ml_dtypes

    BF16 = np.dtype(ml_dtypes.bfloat16)
except ImportError:  # pragma: no cover
    BF16 = None

for _p in ("/opt/trn_rl_repo", os.path.expanduser("~/.axon_site/_ro/trn_rl_repo")):
    if os.path.isdir(_p) and _p not in sys.path:
        sys.path.insert(0, _p)

B, T, D = 2, 2048, 1024
NHEADS, HD, HALF = 16, 64, 32
HPC = 4  # heads per core
N_CORES = 8
ROPE_BASE = 10000.0
SCALE = float(HD) ** -0.5  # 0.125
NDC = D // 128  # 8 contraction chunks for the projections
NKC = T // 128  # 16 k chunks per head
RING_BANKS = 6

_SHUF_MASK = [i ^ 1 for i in range(32)]

_ctx: dict = {}


def _build_nc(iters: int = 0):
    import concourse.bacc as bacc
    import concourse.mybir as mybir
    import concourse.tile as tile

    f32 = mybir.dt.float32
    f32r = mybir.dt.float32r
    bf16 = mybir.dt.bfloat16
    u32 = mybir.dt.uint32
    Exp = mybir.ActivationFunctionType.Exp
    MUL = mybir.AluOpType.mult
    ADD = mybir.AluOpType.add

    nc = bacc.Bacc("TRN2", target_bir_lowering=False, debug=False)

    xt_d = nc.dram_tensor("xt", [D, T], f32, kind="ExternalInput").ap()
    wq_d = nc.dram_tensor("wq", [D, 256], f32, kind="ExternalInput").ap()
    wk_d = nc.dram_tensor("wk", [D, 256], f32, kind="ExternalInput").ap()
    wv_d = nc.dram_tensor("wv", [D, 256], f32, kind="ExternalInput").ap()
    wo_d = nc.dram_tensor("wo", [256, D], bf16, kind="ExternalInput").ap()
    cos_d = nc.dram_tensor("cosb", [128, T], f32, kind="ExternalInput").ap()
    sin_d = nc.dram_tensor("sinb", [128, T], f32, kind="ExternalInput").ap()
    qb_d = nc.dram_tensor("qb", [128, 2], f32, kind="ExternalInput").ap()
    kb_d = nc.dram_tensor("kb", [128, 2], f32, kind="ExternalInput").ap()
    ones_d = nc.dram_tensor("onesd", [128, 4], bf16, kind="ExternalInput").ap()
    y_d = nc.dram_tensor("y", [T, D], f32, kind="ExternalOutput").ap()

    with tile.TileContext(nc) as tc:
        with (
            tc.tile_pool(name="big", bufs=15) as big,
            tc.tile_pool(name="mpool", bufs=2) as mpool,
            tc.tile_pool(name="shpool", bufs=2) as shpool,
            tc.tile_pool(name="opool", bufs=3) as opool,
            tc.tile_pool(name="vpool", bufs=16) as vpool,
            tc.tile_pool(name="small", bufs=2) as small,
            tc.tile_pool(name="ypool", bufs=4) as ypool,
            tc.tile_pool(name="pss", bufs=1, space="PSUM") as pss,
            tc.tile_pool(name="pss2", bufs=1, space="PSUM") as pss2,
            tc.tile_pool(name="psb", bufs=2, space="PSUM") as psb,
        ):

            def body():
                # ---- loads, ordered by first use ----
                def load_w(dram, name):
                    t_ = big.tile([128, 8 * 256], f32r, tag="big", name=name)
                    for ch in range(NDC):
                        nc.sync.dma_start(
                            out=t_[:, ch * 256 : (ch + 1) * 256],
                            in_=dram[ch * 128 : (ch + 1) * 128, :].bitcast(f32r),
                        )
                    return t_

                wq_t = load_w(wq_d, "wq_t")
                xts = []
                for ch in range(NDC):
                    t_ = big.tile([128, T], f32r, tag="big", name=f"xt{ch}")
                    nc.sync.dma_start(
                        out=t_[:], in_=xt_d[ch * 128 : (ch + 1) * 128, :].bitcast(f32r)
                    )
                    xts.append(t_)
                wk_t = load_w(wk_d, "wk_t")
                wv_t = load_w(wv_d, "wv_t")
                qb_t = small.tile([128, 2], f32, tag="bias", name="qb_t")
                nc.scalar.dma_start(out=qb_t[:], in_=qb_d)
                kb_t = small.tile([128, 2], f32, tag="bias", name="kb_t")
                nc.scalar.dma_start(out=kb_t[:], in_=kb_d)
                cos_t = mpool.tile([128, T], f32r, tag="cs", name="cos_t")
                nc.scalar.dma_start(out=cos_t[:], in_=cos_d.bitcast(f32r))
                sin_t = mpool.tile([128, T], f32r, tag="cs", name="sin_t")
                nc.scalar.dma_start(out=sin_t[:], in_=sin_d.bitcast(f32r))

                ringA = pss.tile([128, 3 * 512], f32, tag="ringA", name="ringA")
                ringB = pss2.tile([128, 3 * 512], f32, tag="ringB", name="ringB")

                def ring_slice(bank, width=1):
                    t_ = ringA if bank < 3 else ringB
                    b = bank if bank < 3 else bank - 3
                    return t_[:, b * 512 : (b + width) * 512]

                # ---- Q/K projections; eviction = DVE tensor_scalar (+bias) ----
                def project(w_t, bias_t, name, pr):
                    qt = big.tile([128, T], f32r, tag="big", name=f"{name}{pr}")
                    for ts in range(4):
                        ps = psb.tile(
                            [128, 512], f32, tag="b1", name=f"ps_{name}{pr}_{ts}"
                        )
                        for ch in range(NDC):
                            c0 = ch * 256 + pr * 128
                            nc.tensor.matmul(
                                ps[:],
                                w_t[:, c0 : c0 + 128],
                                xts[ch][:, ts * 512 : (ts + 1) * 512],
                                start=(ch == 0),
                                stop=(ch == NDC - 1),
                            )
                        nc.vector.tensor_scalar_add(
                            qt[:, ts * 512 : (ts + 1) * 512],
                            ps[:],
                            bias_t[:, pr : pr + 1],
                        )
                    return qt

                def rope(t_, name):
                    for si, s0 in enumerate(range(0, T, 1024)):
                        sl = slice(s0, s0 + 1024)
                        sh = shpool.tile([128, 1024], f32r, tag="sh", name=f"sh_{name}{si}")
                        nc.vector.stream_shuffle(
                            sh.bitcast(u32)[:], t_.bitcast(u32)[:, sl], _SHUF_MASK
                        )
                        nc.vector.tensor_tensor(
                            out=t_[:, sl], in0=t_[:, sl], in1=cos_t[:, sl], op=MUL
                        )
                        nc.vector.tensor_tensor(
                            out=sh[:], in0=sh[:], in1=sin_t[:, sl], op=MUL
                        )
                        nc.vector.tensor_tensor(
                            out=t_[:, sl], in0=t_[:, sl], in1=sh[:], op=ADD
                        )

                qts, kts = [], []
                for pr in range(2):
                    qts.append(project(wq_t, qb_t, "qt", pr))
                    kts.append(project(wk_t, kb_t, "kt", pr))
                    rope(qts[pr], f"q{pr}")
                    rope(kts[pr], f"k{pr}")

                # ---- V projection ----
                vts = []
                for tk in range(NKC):
                    vt = vpool.tile([128, HPC * 65], bf16, tag="v", name=f"v{tk}")
                    nc.scalar.dma_start(
                        out=vt.rearrange("p (h c) -> p h c", c=65)[:, :, 64:65],
                        in_=ones_d.rearrange("p (h c) -> p h c", c=1),
                    )
                    ps = psb.tile([128, 256], f32, tag="b1", name=f"psv{tk}")
                    for ch in range(NDC):
                        nc.tensor.matmul(
                            ps[:],
                            xts[ch][:, tk * 128 : (tk + 1) * 128],
                            wv_t[:, ch * 256 : (ch + 1) * 256],
                            start=(ch == 0),
                            stop=(ch == NDC - 1),
                        )
                    nc.vector.tensor_copy(
                        vt.rearrange("p (h c) -> p h c", c=65)[:, :, 0:64],
                        ps.rearrange("p (h c) -> p h c", c=64),
                    )
                    vts.append(vt)

                wo_t = opool.tile([128, 2048], bf16, tag="o", name="wo_t")
                for r in range(2):
                    nc.sync.dma_start(
                        out=wo_t[:, r * 1024 : (r + 1) * 1024],
                        in_=wo_d[r * 128 : (r + 1) * 128, :],
                    )

                # ---- attention (per q-tile, both pairs) + interleaved wo ----
                ot0 = opool.tile([128, T], bf16, tag="o", name="ot0")
                ot1 = opool.tile([128, T], bf16, tag="o", name="ot1")
                ots = [ot0, ot1]
                ROUNDS = [(0, 1, 2), (3, 4, 5), (6, 7, 8), (9, 10, 11), (12, 13, 14), (15,)]
                for qi in range(4):
                    qs = qi * 512
                    for pr in range(2):
                        oaccs = []
                        for hh in range(2):
                            oaccs.append(
                                psb.tile([65, 512], f32, tag="b1", name=f"o_{pr}_{qi}_{hh}")
                            )
                        for ri, rnd in enumerate(ROUNDS):
                            for i, kc in enumerate(rnd):
                                for hh in range(2):
                                    nc.tensor.matmul(
                                        ring_slice(2 * i + hh),
                                        kts[pr][hh * 64 : hh * 64 + 64, kc * 128 : (kc + 1) * 128],
                                        qts[pr][hh * 64 : hh * 64 + 64, qs : qs + 512],
                                        start=True,
                                        stop=True,
                                    )
                            nb = 2 * len(rnd)
                            for g0 in range(0, nb, 3):
                                gw = min(3, nb - g0)
                                pt = mpool.tile(
                                    [128, 512 * gw],
                                    bf16,
                                    tag="p",
                                    bufs=8,
                                    name=f"p_{pr}_{qi}_{ri}_{g0}",
                                )
                                nc.scalar.activation(
                                    pt[:],
                                    ring_slice(g0, gw),
                                    Exp,
                                    bias=0.0,
                                    scale=SCALE,
                                )
                                for j in range(gw):
                                    i, hh = divmod(g0 + j, 2)
                                    kc = rnd[i]
                                    nc.tensor.matmul(
                                        oaccs[hh][:],
                                        vts[kc][:, (2 * pr + hh) * 65 : (2 * pr + hh + 1) * 65],
                                        pt[:, j * 512 : (j + 1) * 512],
                                        start=(kc == 0),
                                        stop=(kc == NKC - 1),
                                    )
                        for hh in range(2):
                            rt = small.tile([1, 512], f32, tag="rt", name=f"rt_{pr}_{qi}_{hh}")
                            nc.vector.reciprocal(rt[0:1, :], oaccs[hh][64:65, :])
                            rb = small.tile([64, 512], f32, tag="rb", name=f"rb_{pr}_{qi}_{hh}")
                            nc.gpsimd.partition_broadcast(rb[:, :], rt[0:1, :], channels=64)
                            nc.vector.tensor_tensor(
                                out=ots[pr][hh * 64 : hh * 64 + 64, qs : qs + 512],
                                in0=oaccs[hh][0:64, :],
                                in1=rb[:, :],
                                op=MUL,
                            )
                    # wo for the 4 t-chunks of this q-tile (Y psum on psb)
                    for tt in range(4 * qi, 4 * qi + 4):
                        for eh in range(2):
                            yps = psb.tile([128, 512], f32, tag="b1", name=f"yp_{tt}_{eh}")
                            for r in range(2):
                                nc.tensor.matmul(
                                    yps[:],
                                    ots[r][:, tt * 128 : (tt + 1) * 128],
                                    wo_t[:, r * 1024 + eh * 512 : r * 1024 + (eh + 1) * 512],
                                    start=(r == 0),
                                    stop=(r == 1),
                                )
                            ysb = ypool.tile([128, 512], f32, tag="y", name=f"y_{tt}_{eh}")
                            nc.vector.tensor_copy(ysb[:], yps[:])
                            nc.sync.dma_start(
                                out=y_d[tt * 128 : (tt + 1) * 128, eh * 512 : (eh + 1) * 512],
                                in_=ysb[:],
                            )

            if iters:
                import concourse.mybir as _mb
                with tc.For_i(
                    0,
                    iters,
                    1,
                    hint_engines=(
                        _mb.EngineType.PE,
                        _mb.EngineType.Activation,
                        _mb.EngineType.DVE,
                        _mb.EngineType.SP,
                        _mb.EngineType.Pool,
                    ),
                ) as _iv:
                    body()
            else:
                body()

    nc.compile()
    return nc


def _host_inputs(x, wq_w, wq_b, wk_w, wk_b, wv_w, wv_b, wo_w, wo_b):
    """Build the 8 per-core input maps (all host-side slicing/transposes)."""
    f = np.float32
    x = np.asarray(x, f)
    wq_w = np.asarray(wq_w, f)
    wk_w = np.asarray(wk_w, f)
    wv_w = np.asarray(wv_w, f)
    wo_w = np.asarray(wo_w, f)
    wq_b = np.asarray(wq_b, f)
    wk_b = np.asarray(wk_b, f)
    wv_b = np.asarray(wv_b, f)
    wo_b = np.asarray(wo_b, f)

    # RoPE tables in fp32, mirroring the reference formulas.
    pos = np.arange(T, dtype=f)[:, None]
    idx = np.arange(HALF, dtype=f)[None, :]
    inv_freq = (f(1.0) / (f(ROPE_BASE) ** (idx / f(HALF)))).astype(f)
    ang = pos * inv_freq  # [T, 32]
    cosv, sinv = np.cos(ang).astype(f), np.sin(ang).astype(f)
    cos64 = np.repeat(cosv.T, 2, axis=0)  # [64, T]
    sin64 = np.repeat(sinv.T, 2, axis=0)
    sin64[0::2] *= -1  # rows 2j: -sin, rows 2j+1: +sin
    cos128 = np.ascontiguousarray(np.tile(cos64, (2, 1)))
    sin128 = np.ascontiguousarray(np.tile(sin64, (2, 1)))

    perm64 = np.empty(64, np.int64)
    perm64[0::2] = np.arange(32)
    perm64[1::2] = np.arange(32) + 32

    xts = [np.ascontiguousarray(x[b].T) for b in range(B)]
    ones_pad = np.ones((128, HPC), BF16)

    in_maps = []
    for c in range(N_CORES):
        b, g = c // 4, c % 4
        heads = np.arange(4 * g, 4 * g + 4)
        qk_cols = np.concatenate([h * 64 + perm64 for h in heads])
        v_cols = np.concatenate([np.arange(h * 64, (h + 1) * 64) for h in heads])
        in_maps.append(
            {
                "xt": xts[b],
                "wq": np.ascontiguousarray(wq_w[:, qk_cols]),
                "wk": np.ascontiguousarray(wk_w[:, qk_cols]),
                "wv": np.ascontiguousarray(wv_w[:, v_cols]),
                "wo": np.ascontiguousarray(wo_w[v_cols, :].astype(BF16)),
                "cosb": cos128,
                "sinb": sin128,
                "qb": np.ascontiguousarray(wq_b[qk_cols].reshape(2, 128).T),
                "kb": np.ascontiguousarray(wk_b[qk_cols].reshape(2, 128).T),
                "onesd": ones_pad,
            }
        )

    beff = (wo_b.astype(np.float64) + wv_b.astype(np.float64) @ wo_w.astype(np.float64)).astype(f)
    return in_maps, beff


def kernel(x, wq_w, wq_b, wk_w, wk_b, wv_w, wv_b, wo_w, wo_b):
    from concourse import bass2jax

    in_maps, beff = _host_inputs(
        x, wq_w, wq_b, wk_w, wk_b, wv_w, wv_b, wo_w, wo_b
    )
    if "nc" not in _ctx:
        _ctx["nc"] = _build_nc(0)
    res = bass2jax.run_bass_via_pjrt(_ctx["nc"], in_maps, n_cores=N_CORES)
    y = np.empty((B, T, D), np.float32)
    for b in range(B):
        acc = res[4 * b]["y"].copy()
        for g in range(1, 4):
            acc += res[4 * b + g]["y"]
        y[b] = acc + beff[None, :]
    return y


# revision 9
# speedup vs baseline: 1.4333x; 1.1073x over previous
# Multi-head self-attention with RoPE on 8 Trainium2 NeuronCores.
#
# Sharding: batch x head-group. Core c handles batch b = c//4 and heads
# 4*(c%4) .. 4*(c%4)+3 (4 of 16 heads). Each core computes Q/K/V
# projections for its heads from the full (transposed) x[b], runs
# attention, and produces a partial output projection
# Y_partial = O_core^T.T @ Wo[rows-of-its-heads]. The host sums the four
# partials per batch and adds the (constant) bias terms.
#
# All matmuls run in float32r (TF32-like, ~1.5e-4 rel rounding) which
# streams at 1 column/cycle on the PE (4x faster than true fp32).
#
# Per-core layouts:
#   XT    [1024 D, 2048 t]  (host-transposed x[b])
#   QT/KT [128 = 2 heads x 64 d(permuted), 2048 t]  x 2 pair-tiles
#   V     16 tiles [128 t-chunk, 4 heads x 65] (65th col = 1.0 -> row sums)
#   S^T   6-bank PSUM ring [128, 3072]; K=64 matmuls for the two heads of a
#         pair run CONCURRENTLY on the PE via row-group tiling (partition
#         bases 0/64), exp'd in groups of 3 banks -> P tiles (f32r)
#   O^T   psum [65, 512]: rows 0..63 = sum_k exp*v, row 64 = Z (denominator)
#   Y     [2048 t, 1024 e] fp32 partial; Y psum reuses the ring banks
#
# RoPE: head-dim rows are pair-interleaved (d' = [0,32,1,33,...]) via a host
# permutation of Wq/Wk columns so the rotate-half partner lives on the
# adjacent partition; a DVE stream_shuffle (pair swap) + 2 muls + 1 add
# apply the rotation with band-replicated, sign-baked cos/sin tables.

import os
import sys

import numpy as np

try:
    import ml_dtypes

    BF16 = np.dtype(ml_dtypes.bfloat16)
except ImportError:  # pragma: no cover
    BF16 = None

for _p in ("/opt/trn_rl_repo", os.path.expanduser("~/.axon_site/_ro/trn_rl_repo")):
    if os.path.isdir(_p) and _p not in sys.path:
        sys.path.insert(0, _p)

B, T, D = 2, 2048, 1024
NHEADS, HD, HALF = 16, 64, 32
HPC = 4  # heads per core
N_CORES = 8
ROPE_BASE = 10000.0
SCALE = float(HD) ** -0.5  # 0.125
NDC = D // 128  # 8 contraction chunks for the projections
NKC = T // 128  # 16 k chunks per head
RING_BANKS = 6

_SHUF_MASK = [i ^ 1 for i in range(32)]

_ctx: dict = {}


def _build_nc(iters: int = 0, phase: str = "full"):
    import concourse.bacc as bacc
    import concourse.mybir as mybir
    import concourse.tile as tile

    f32 = mybir.dt.float32
    f32r = mybir.dt.float32r
    bf16 = mybir.dt.bfloat16
    u32 = mybir.dt.uint32
    Exp = mybir.ActivationFunctionType.Exp
    MUL = mybir.AluOpType.mult
    ADD = mybir.AluOpType.add

    nc = bacc.Bacc("TRN2", target_bir_lowering=False, debug=False)

    xt_d = nc.dram_tensor("xt", [D, T], f32, kind="ExternalInput").ap()
    wq_d = nc.dram_tensor("wq", [D, 256], f32, kind="ExternalInput").ap()
    wk_d = nc.dram_tensor("wk", [D, 256], f32, kind="ExternalInput").ap()
    wv_d = nc.dram_tensor("wv", [D, 256], f32, kind="ExternalInput").ap()
    wo_d = nc.dram_tensor("wo", [256, D], bf16, kind="ExternalInput").ap()
    cos_d = nc.dram_tensor("cosb", [128, T], f32, kind="ExternalInput").ap()
    sin_d = nc.dram_tensor("sinb", [128, T], f32, kind="ExternalInput").ap()
    qb_d = nc.dram_tensor("qb", [128, 2], f32, kind="ExternalInput").ap()
    kb_d = nc.dram_tensor("kb", [128, 2], f32, kind="ExternalInput").ap()
    ones_d = nc.dram_tensor("onesd", [128, 4], bf16, kind="ExternalInput").ap()
    y_d = nc.dram_tensor("y", [T, D], f32, kind="ExternalOutput").ap()

    with tile.TileContext(nc) as tc:
        with (
            tc.tile_pool(name="big", bufs=15) as big,
            tc.tile_pool(name="mpool", bufs=2) as mpool,
            tc.tile_pool(name="shpool", bufs=2) as shpool,
            tc.tile_pool(name="opool", bufs=3) as opool,
            tc.tile_pool(name="vpool", bufs=16) as vpool,
            tc.tile_pool(name="small", bufs=2) as small,
            tc.tile_pool(name="ypool", bufs=4) as ypool,
            tc.tile_pool(name="pss", bufs=1, space="PSUM") as pss,
            tc.tile_pool(name="pss2", bufs=1, space="PSUM") as pss2,
            tc.tile_pool(name="psb", bufs=2, space="PSUM") as psb,
        ):

            def body():
                # ---- loads, ordered by first use ----
                def load_w(dram, name):
                    t_ = big.tile([128, 8 * 256], f32r, tag="big", name=name)
                    for ch in range(NDC):
                        nc.sync.dma_start(
                            out=t_[:, ch * 256 : (ch + 1) * 256],
                            in_=dram[ch * 128 : (ch + 1) * 128, :].bitcast(f32r),
                        )
                    return t_

                wq_t = load_w(wq_d, "wq_t")
                xts = []
                for ch in range(NDC):
                    t_ = big.tile([128, T], f32r, tag="big", name=f"xt{ch}")
                    nc.sync.dma_start(
                        out=t_[:], in_=xt_d[ch * 128 : (ch + 1) * 128, :].bitcast(f32r)
                    )
                    xts.append(t_)
                wk_t = load_w(wk_d, "wk_t")
                wv_t = load_w(wv_d, "wv_t")
                qb_t = small.tile([128, 2], f32, tag="bias", name="qb_t")
                nc.scalar.dma_start(out=qb_t[:], in_=qb_d)
                kb_t = small.tile([128, 2], f32, tag="bias", name="kb_t")
                nc.scalar.dma_start(out=kb_t[:], in_=kb_d)
                cos_t = mpool.tile([128, T], f32r, tag="cs", name="cos_t")
                nc.scalar.dma_start(out=cos_t[:], in_=cos_d.bitcast(f32r))
                sin_t = mpool.tile([128, T], f32r, tag="cs", name="sin_t")
                nc.scalar.dma_start(out=sin_t[:], in_=sin_d.bitcast(f32r))

                ringA = pss.tile([128, 3 * 512], f32, tag="ringA", name="ringA")
                ringB = pss2.tile([128, 3 * 512], f32, tag="ringB", name="ringB")

                def ring_slice(bank, width=1):
                    t_ = ringA if bank < 3 else ringB
                    b = bank if bank < 3 else bank - 3
                    return t_[:, b * 512 : (b + width) * 512]

                # ---- Q/K projections; eviction = DVE tensor_scalar (+bias) ----
                def project(w_t, bias_t, name, pr):
                    qt = big.tile([128, T], f32r, tag="big", name=f"{name}{pr}")
                    for ts in range(4):
                        ps = psb.tile(
                            [128, 512], f32, tag="b1", name=f"ps_{name}{pr}_{ts}"
                        )
                        for ch in range(NDC):
                            c0 = ch * 256 + pr * 128
                            nc.tensor.matmul(
                                ps[:],
                                w_t[:, c0 : c0 + 128],
                                xts[ch][:, ts * 512 : (ts + 1) * 512],
                                start=(ch == 0),
                                stop=(ch == NDC - 1),
                            )
                        nc.vector.tensor_scalar_add(
                            qt[:, ts * 512 : (ts + 1) * 512],
                            ps[:],
                            bias_t[:, pr : pr + 1],
                        )
                    return qt

                def rope(t_, name):
                    for si, s0 in enumerate(range(0, T, 1024)):
                        sl = slice(s0, s0 + 1024)
                        sh = shpool.tile([128, 1024], f32r, tag="sh", name=f"sh_{name}{si}")
                        nc.vector.stream_shuffle(
                            sh.bitcast(u32)[:], t_.bitcast(u32)[:, sl], _SHUF_MASK
                        )
                        nc.vector.tensor_tensor(
                            out=t_[:, sl], in0=t_[:, sl], in1=cos_t[:, sl], op=MUL
                        )
                        nc.vector.tensor_tensor(
                            out=sh[:], in0=sh[:], in1=sin_t[:, sl], op=MUL
                        )
                        nc.vector.tensor_tensor(
                            out=t_[:, sl], in0=t_[:, sl], in1=sh[:], op=ADD
                        )

                qts, kts = [], []
                for pr in range(2):
                    qts.append(project(wq_t, qb_t, "qt", pr))
                    kts.append(project(wk_t, kb_t, "kt", pr))
                    rope(qts[pr], f"q{pr}")
                    rope(kts[pr], f"k{pr}")

                # ---- V projection ----
                vts = []
                for tk in range(NKC):
                    vt = vpool.tile([128, HPC * 65], bf16, tag="v", name=f"v{tk}")
                    nc.scalar.dma_start(
                        out=vt.rearrange("p (h c) -> p h c", c=65)[:, :, 64:65],
                        in_=ones_d.rearrange("p (h c) -> p h c", c=1),
                    )
                    ps = psb.tile([128, 256], f32, tag="b1", name=f"psv{tk}")
                    for ch in range(NDC):
                        nc.tensor.matmul(
                            ps[:],
                            xts[ch][:, tk * 128 : (tk + 1) * 128],
                            wv_t[:, ch * 256 : (ch + 1) * 256],
                            start=(ch == 0),
                            stop=(ch == NDC - 1),
                        )
                    nc.vector.tensor_copy(
                        vt.rearrange("p (h c) -> p h c", c=65)[:, :, 0:64],
                        ps.rearrange("p (h c) -> p h c", c=64),
                    )
                    vts.append(vt)

                wo_t = opool.tile([128, 2048], bf16, tag="o", name="wo_t")
                for r in range(2):
                    nc.sync.dma_start(
                        out=wo_t[:, r * 1024 : (r + 1) * 1024],
                        in_=wo_d[r * 128 : (r + 1) * 128, :],
                    )

                if phase == "proj":
                    dbg = ypool.tile([128, 512], f32, tag="y", name="dbg")
                    nc.vector.tensor_copy(dbg[:], qts[0].bitcast(f32)[:, 0:512])
                    nc.sync.dma_start(out=y_d[0:128, 0:512], in_=dbg[:])
                    return

                # ---- attention (per q-tile, both pairs) + interleaved wo ----
                ot0 = opool.tile([128, T], bf16, tag="o", name="ot0")
                ot1 = opool.tile([128, T], bf16, tag="o", name="ot1")
                ots = [ot0, ot1]
                ROUNDS = [(0, 1, 2), (3, 4, 5), (6, 7, 8), (9, 10, 11), (12, 13, 14), (15,)]
                for qi in range(4):
                    qs = qi * 512
                    for pr in range(2):
                        oaccs = []
                        for hh in range(2):
                            oaccs.append(
                                psb.tile([65, 512], f32, tag="b1", name=f"o_{pr}_{qi}_{hh}")
                            )
                        for ri, rnd in enumerate(ROUNDS):
                            for i, kc in enumerate(rnd):
                                for hh in range(2):
                                    nc.tensor.matmul(
                                        ring_slice(2 * i + hh),
                                        kts[pr][hh * 64 : hh * 64 + 64, kc * 128 : (kc + 1) * 128],
                                        qts[pr][hh * 64 : hh * 64 + 64, qs : qs + 512],
                                        start=True,
                                        stop=True,
                                    )
                            nb = 2 * len(rnd)
                            for g0 in range(0, nb, 3):
                                gw = min(3, nb - g0)
                                pt = mpool.tile(
                                    [128, 512 * gw],
                                    bf16,
                                    tag="p",
                                    bufs=8,
                                    name=f"p_{pr}_{qi}_{ri}_{g0}",
                                )
                                nc.scalar.activation(
                                    pt[:],
                                    ring_slice(g0, gw),
                                    Exp,
                                    bias=0.0,
                                    scale=SCALE,
                                )
                                for j in range(gw):
                                    i, hh = divmod(g0 + j, 2)
                                    kc = rnd[i]
                                    nc.tensor.matmul(
                                        oaccs[hh][:],
                                        vts[kc][:, (2 * pr + hh) * 65 : (2 * pr + hh + 1) * 65],
                                        pt[:, j * 512 : (j + 1) * 512],
                                        start=(kc == 0),
                                        stop=(kc == NKC - 1),
                                    )
                        for hh in range(2):
                            rt = small.tile([1, 512], f32, tag="rt", name=f"rt_{pr}_{qi}_{hh}")
                            nc.vector.reciprocal(rt[0:1, :], oaccs[hh][64:65, :])
                            rb = small.tile([64, 512], f32, tag="rb", name=f"rb_{pr}_{qi}_{hh}")
                            nc.gpsimd.partition_broadcast(rb[:, :], rt[0:1, :], channels=64)
                            nc.vector.tensor_tensor(
                                out=ots[pr][hh * 64 : hh * 64 + 64, qs : qs + 512],
                                in0=oaccs[hh][0:64, :],
                                in1=rb[:, :],
                                op=MUL,
                            )
                    # wo for the 4 t-chunks of this q-tile (Y psum on psb)
                    for tt in ([] if phase == "attn" else range(4 * qi, 4 * qi + 4)):
                        for eh in range(2):
                            yps = psb.tile([128, 512], f32, tag="b1", name=f"yp_{tt}_{eh}")
                            for r in range(2):
                                nc.tensor.matmul(
                                    yps[:],
                                    ots[r][:, tt * 128 : (tt + 1) * 128],
                                    wo_t[:, r * 1024 + eh * 512 : r * 1024 + (eh + 1) * 512],
                                    start=(r == 0),
                                    stop=(r == 1),
                                )
                            ysb = ypool.tile([128, 512], f32, tag="y", name=f"y_{tt}_{eh}")
                            nc.vector.tensor_copy(ysb[:], yps[:])
                            nc.sync.dma_start(
                                out=y_d[tt * 128 : (tt + 1) * 128, eh * 512 : (eh + 1) * 512],
                                in_=ysb[:],
                            )

                if phase == "attn":
                    dbg2 = ypool.tile([128, 512], f32, tag="y", name="dbg2")
                    nc.vector.tensor_copy(dbg2[:], ots[0].bitcast(f32)[:, 0:256])
                    nc.sync.dma_start(out=y_d[0:128, 0:512], in_=dbg2[:])

            if iters:
                import concourse.mybir as _mb
                with tc.For_i(
                    0,
                    iters,
                    1,
                    hint_engines=(
                        _mb.EngineType.PE,
                        _mb.EngineType.Activation,
                        _mb.EngineType.DVE,
                        _mb.EngineType.SP,
                        _mb.EngineType.Pool,
                    ),
                    staggered_reset=True,
                ) as _iv:
                    body()
            else:
                body()

    nc.compile()
    return nc


def _host_inputs(x, wq_w, wq_b, wk_w, wk_b, wv_w, wv_b, wo_w, wo_b):
    """Build the 8 per-core input maps (all host-side slicing/transposes)."""
    f = np.float32
    x = np.asarray(x, f)
    wq_w = np.asarray(wq_w, f)
    wk_w = np.asarray(wk_w, f)
    wv_w = np.asarray(wv_w, f)
    wo_w = np.asarray(wo_w, f)
    wq_b = np.asarray(wq_b, f)
    wk_b = np.asarray(wk_b, f)
    wv_b = np.asarray(wv_b, f)
    wo_b = np.asarray(wo_b, f)

    # RoPE tables in fp32, mirroring the reference formulas.
    pos = np.arange(T, dtype=f)[:, None]
    idx = np.arange(HALF, dtype=f)[None, :]
    inv_freq = (f(1.0) / (f(ROPE_BASE) ** (idx / f(HALF)))).astype(f)
    ang = pos * inv_freq  # [T, 32]
    cosv, sinv = np.cos(ang).astype(f), np.sin(ang).astype(f)
    cos64 = np.repeat(cosv.T, 2, axis=0)  # [64, T]
    sin64 = np.repeat(sinv.T, 2, axis=0)
    sin64[0::2] *= -1  # rows 2j: -sin, rows 2j+1: +sin
    cos128 = np.ascontiguousarray(np.tile(cos64, (2, 1)))
    sin128 = np.ascontiguousarray(np.tile(sin64, (2, 1)))

    perm64 = np.empty(64, np.int64)
    perm64[0::2] = np.arange(32)
    perm64[1::2] = np.arange(32) + 32

    xts = [np.ascontiguousarray(x[b].T) for b in range(B)]
    ones_pad = np.ones((128, HPC), BF16)

    in_maps = []
    for c in range(N_CORES):
        b, g = c // 4, c % 4
        heads = np.arange(4 * g, 4 * g + 4)
        qk_cols = np.concatenate([h * 64 + perm64 for h in heads])
        v_cols = np.concatenate([np.arange(h * 64, (h + 1) * 64) for h in heads])
        in_maps.append(
            {
                "xt": xts[b],
                "wq": np.ascontiguousarray(wq_w[:, qk_cols]),
                "wk": np.ascontiguousarray(wk_w[:, qk_cols]),
                "wv": np.ascontiguousarray(wv_w[:, v_cols]),
                "wo": np.ascontiguousarray(wo_w[v_cols, :].astype(BF16)),
                "cosb": cos128,
                "sinb": sin128,
                "qb": np.ascontiguousarray(wq_b[qk_cols].reshape(2, 128).T),
                "kb": np.ascontiguousarray(wk_b[qk_cols].reshape(2, 128).T),
                "onesd": ones_pad,
            }
        )

    beff = (wo_b.astype(np.float64) + wv_b.astype(np.float64) @ wo_w.astype(np.float64)).astype(f)
    return in_maps, beff


def kernel(x, wq_w, wq_b, wk_w, wk_b, wv_w, wv_b, wo_w, wo_b):
    from concourse import bass2jax

    in_maps, beff = _host_inputs(
        x, wq_w, wq_b, wk_w, wk_b, wv_w, wv_b, wo_w, wo_b
    )
    if "nc" not in _ctx:
        _ctx["nc"] = _build_nc(0)
    res = bass2jax.run_bass_via_pjrt(_ctx["nc"], in_maps, n_cores=N_CORES)
    y = np.empty((B, T, D), np.float32)
    for b in range(B):
        acc = res[4 * b]["y"].copy()
        for g in range(1, 4):
            acc += res[4 * b + g]["y"]
        y[b] = acc + beff[None, :]
    return y


# revision 12
# speedup vs baseline: 1.5829x; 1.1044x over previous
# Multi-head self-attention with RoPE on 8 Trainium2 NeuronCores.
#
# Sharding: batch x head-group. Core c handles batch b = c//4 and heads
# 4*(c%4) .. 4*(c%4)+3 (4 of 16 heads). Each core computes Q/K/V
# projections for its heads from the full (transposed) x[b], runs
# attention, and produces a partial output projection
# Y_partial = O_core^T.T @ Wo[rows-of-its-heads]. The host sums the four
# partials per batch and adds the (constant) bias terms.
#
# Matmul dtypes: projections and S = Q.K^T run in float32r (TF32-like,
# ~1.5e-4 rel rounding, 1 column/cycle on the PE — 4x faster than true
# fp32). The post-softmax path (P, V, O, Wo) runs in bf16 with fp32 PSUM
# accumulation.
#
# Per-core layouts:
#   XT    [128, 8 x 2048]   x[b]^T chunk-packed (host-side), 2 tiles
#   QT/KT [128 = 2 heads x 64 d(permuted), 2048 t]  x 2 pair-tiles, f32r
#   V     16 tiles [128 t-chunk, 4 heads x 65] bf16 (65th col = 1.0)
#   S^T   two 3-bank PSUM groups [128, 1536]; K=64 matmuls for the two
#         heads of a pair run CONCURRENTLY on the PE via row-group tiling
#         (partition bases 0/64); exp'd per 3-bank group -> P tiles (bf16)
#   O^T   psum [65, 512]: rows 0..63 = sum_k exp*v, row 64 = Z (denominator)
#   Y     [2048 t, 1024 e] fp32 partial, evicted in [128, 2048] tiles
#
# DMA discipline: per-dma_start overhead on a HWDGE ring is ~4-5us, so all
# inputs are packed host-side into 5 large transfers split across the two
# rings (SP + ACT); outputs go out as 8 x 1MB transfers.
#
# RoPE: head-dim rows are pair-interleaved (d' = [0,32,1,33,...]) via a host
# permutation of Wq/Wk columns so the rotate-half partner lives on the
# adjacent partition; a DVE stream_shuffle (pair swap) + 2 muls + 1 add
# apply the rotation with band-replicated, sign-baked cos/sin tables.

import os
import sys

import numpy as np

try:
    import ml_dtypes

    BF16 = np.dtype(ml_dtypes.bfloat16)
except ImportError:  # pragma: no cover
    BF16 = None

for _p in ("/opt/trn_rl_repo", os.path.expanduser("~/.axon_site/_ro/trn_rl_repo")):
    if os.path.isdir(_p) and _p not in sys.path:
        sys.path.insert(0, _p)

B, T, D = 2, 2048, 1024
NHEADS, HD, HALF = 16, 64, 32
HPC = 4  # heads per core
N_CORES = 8
ROPE_BASE = 10000.0
SCALE = float(HD) ** -0.5  # 0.125
NDC = D // 128  # 8 contraction chunks for the projections
NKC = T // 128  # 16 k chunks per head

_SHUF_MASK = [i ^ 1 for i in range(32)]

_ctx: dict = {}


def _build_nc(iters: int = 0, phase: str = "full"):
    import concourse.bacc as bacc
    import concourse.mybir as mybir
    import concourse.tile as tile

    f32 = mybir.dt.float32
    f32r = mybir.dt.float32r
    bf16 = mybir.dt.bfloat16
    u32 = mybir.dt.uint32
    Exp = mybir.ActivationFunctionType.Exp
    MUL = mybir.AluOpType.mult
    ADD = mybir.AluOpType.add

    nc = bacc.Bacc("TRN2", target_bir_lowering=False, debug=False)

    # packed inputs (see _host_inputs for layouts)
    xt_d = nc.dram_tensor("xtp", [128, NDC * T], f32, kind="ExternalInput").ap()
    wp1_d = nc.dram_tensor("wp1", [128, 4096], f32, kind="ExternalInput").ap()
    wp2_d = nc.dram_tensor("wp2", [128, 6148], f32, kind="ExternalInput").ap()
    wo_d = nc.dram_tensor("wop", [128, 2048], bf16, kind="ExternalInput").ap()
    y_d = nc.dram_tensor("y", [T, D], f32, kind="ExternalOutput").ap()
    y_r = y_d.rearrange("(b p) e -> p b e", p=128)  # [128, 16, 1024]

    with tile.TileContext(nc) as tc:
        with (
            tc.tile_pool(name="xtpool", bufs=2) as xtpool,
            tc.tile_pool(name="wpool", bufs=1) as wpool,
            tc.tile_pool(name="qkpool", bufs=4) as qkpool,
            tc.tile_pool(name="ppool", bufs=6) as ppool,
            tc.tile_pool(name="shpool", bufs=2) as shpool,
            tc.tile_pool(name="opool", bufs=3) as opool,
            tc.tile_pool(name="vpool", bufs=16) as vpool,
            tc.tile_pool(name="small", bufs=2) as small,
            tc.tile_pool(name="ypool", bufs=2) as ypool,
            tc.tile_pool(name="pss", bufs=1, space="PSUM") as pss,
            tc.tile_pool(name="pss2", bufs=1, space="PSUM") as pss2,
            tc.tile_pool(name="psb", bufs=2, space="PSUM") as psb,
        ):

            def body():
                # ---- 5 packed input DMAs across both HWDGE rings ----
                wp1 = wpool.tile([128, 4096], f32r, tag="wp1", name="wp1")
                nc.sync.dma_start(out=wp1[:], in_=wp1_d.bitcast(f32r))
                xta = xtpool.tile([128, 4 * T], f32r, tag="xt", name="xta")
                nc.sync.dma_start(out=xta[:], in_=xt_d[:, 0 : 4 * T].bitcast(f32r))
                wp2 = wpool.tile([128, 6148], f32r, tag="wp2", name="wp2")
                nc.scalar.dma_start(out=wp2[:], in_=wp2_d.bitcast(f32r))
                xtb = xtpool.tile([128, 4 * T], f32r, tag="xt", name="xtb")
                nc.scalar.dma_start(out=xtb[:], in_=xt_d[:, 4 * T :].bitcast(f32r))
                wo_t = opool.tile([128, 2048], bf16, tag="o", name="wo_t")
                nc.sync.dma_start(out=wo_t[:], in_=wo_d)

                def xt_sl(ch, lo, hi):
                    t_ = xta if ch < 4 else xtb
                    return t_[:, (ch % 4) * T + lo : (ch % 4) * T + hi]

                def wq_sl(ch, pr):  # [128 Dchunk, 128]
                    c0 = ch * 256 + pr * 128
                    return wp1[:, c0 : c0 + 128]

                def wk_sl(ch, pr):
                    c0 = 2048 + ch * 256 + pr * 128
                    return wp1[:, c0 : c0 + 128]

                def wv_sl(ch):  # [128, 256]
                    return wp2[:, ch * 256 : (ch + 1) * 256]

                cos_t = wp2[:, 2048:4096]
                sin_t = wp2[:, 4096:6144]
                qb_t = wp2.bitcast(f32)[:, 6144:6146]
                kb_t = wp2.bitcast(f32)[:, 6146:6148]

                ringA = pss.tile([128, 3 * 512], f32, tag="ringA", name="ringA")
                ringB = pss2.tile([128, 3 * 512], f32, tag="ringB", name="ringB")

                def ring_slice(bank, width=1):
                    t_ = ringA if bank < 3 else ringB
                    b = bank if bank < 3 else bank - 3
                    return t_[:, b * 512 : (b + width) * 512]

                # ---- Q/K projections; eviction = DVE tensor_scalar (+bias) ----
                def project(w_sl, bias_t, name, pr):
                    qt = qkpool.tile([128, T], f32r, tag="qk", name=f"{name}{pr}")
                    for ts in range(4):
                        ps = psb.tile(
                            [128, 512], f32, tag="b1", name=f"ps_{name}{pr}_{ts}"
                        )
                        for ch in range(NDC):
                            nc.tensor.matmul(
                                ps[:],
                                w_sl(ch, pr),
                                xt_sl(ch, ts * 512, (ts + 1) * 512),
                                start=(ch == 0),
                                stop=(ch == NDC - 1),
                            )
                        nc.vector.tensor_scalar_add(
                            qt[:, ts * 512 : (ts + 1) * 512],
                            ps[:],
                            bias_t[:, pr : pr + 1],
                        )
                    return qt

                def rope(t_, name):
                    for si, s0 in enumerate(range(0, T, 1024)):
                        sl = slice(s0, s0 + 1024)
                        sh = shpool.tile([128, 1024], f32r, tag="sh", name=f"sh_{name}{si}")
                        nc.vector.stream_shuffle(
                            sh.bitcast(u32)[:], t_.bitcast(u32)[:, sl], _SHUF_MASK
                        )
                        nc.vector.tensor_tensor(
                            out=t_[:, sl], in0=t_[:, sl], in1=cos_t[:, sl], op=MUL
                        )
                        nc.vector.tensor_tensor(
                            out=sh[:], in0=sh[:], in1=sin_t[:, sl], op=MUL
                        )
                        nc.vector.tensor_tensor(
                            out=t_[:, sl], in0=t_[:, sl], in1=sh[:], op=ADD
                        )

                qts, kts = [], []
                for pr in range(2):
                    qts.append(project(wq_sl, qb_t, "qt", pr))
                    kts.append(project(wk_sl, kb_t, "kt", pr))
                    rope(qts[pr], f"q{pr}")
                    rope(kts[pr], f"k{pr}")

                # ---- V projection (bf16 tiles, ones col via memset) ----
                vts = []
                for tk in range(NKC):
                    vt = vpool.tile([128, HPC * 65], bf16, tag="v", name=f"v{tk}")
                    nc.vector.memset(
                        vt.rearrange("p (h c) -> p h c", c=65)[:, :, 64:65], 1.0
                    )
                    ps = psb.tile([128, 256], f32, tag="b1", name=f"psv{tk}")
                    for ch in range(NDC):
                        nc.tensor.matmul(
                            ps[:],
                            xt_sl(ch, tk * 128, (tk + 1) * 128),
                            wv_sl(ch),
                            start=(ch == 0),
                            stop=(ch == NDC - 1),
                        )
                    nc.vector.tensor_copy(
                        vt.rearrange("p (h c) -> p h c", c=65)[:, :, 0:64],
                        ps.rearrange("p (h c) -> p h c", c=64),
                    )
                    vts.append(vt)

                if phase == "proj":
                    dbg = ypool.tile([128, 2048], f32, tag="y", name="dbg")
                    nc.vector.tensor_copy(dbg[:, 0:512], qts[0].bitcast(f32)[:, 0:512])
                    nc.sync.dma_start(
                        out=y_r[:, 0:2, :], in_=dbg.rearrange("p (b e) -> p b e", e=1024)
                    )
                    return

                # ---- attention (per q-tile, both pairs) + interleaved wo ----
                ot0 = opool.tile([128, T], bf16, tag="o", name="ot0")
                ot1 = opool.tile([128, T], bf16, tag="o", name="ot1")
                ots = [ot0, ot1]
                ROUNDS = [(0, 1, 2), (3, 4, 5), (6, 7, 8), (9, 10, 11), (12, 13, 14), (15,)]
                for qi in range(4):
                    qs = qi * 512
                    for pr in range(2):
                        oaccs = []
                        for hh in range(2):
                            oaccs.append(
                                psb.tile([65, 512], f32, tag="b1", name=f"o_{pr}_{qi}_{hh}")
                            )
                        for ri, rnd in enumerate(ROUNDS):
                            for i, kc in enumerate(rnd):
                                for hh in range(2):
                                    nc.tensor.matmul(
                                        ring_slice(2 * i + hh),
                                        kts[pr][hh * 64 : hh * 64 + 64, kc * 128 : (kc + 1) * 128],
                                        qts[pr][hh * 64 : hh * 64 + 64, qs : qs + 512],
                                        start=True,
                                        stop=True,
                                    )
                            nb = 2 * len(rnd)
                            for g0 in range(0, nb, 3):
                                gw = min(3, nb - g0)
                                pt = ppool.tile(
                                    [128, 512 * gw],
                                    bf16,
                                    tag="p",
                                    name=f"p_{pr}_{qi}_{ri}_{g0}",
                                )
                                nc.scalar.activation(
                                    pt[:],
                                    ring_slice(g0, gw),
                                    Exp,
                                    bias=0.0,
                                    scale=SCALE,
                                )
                                for j in range(gw):
                                    i, hh = divmod(g0 + j, 2)
                                    kc = rnd[i]
                                    nc.tensor.matmul(
                                        oaccs[hh][:],
                                        vts[kc][:, (2 * pr + hh) * 65 : (2 * pr + hh + 1) * 65],
                                        pt[:, j * 512 : (j + 1) * 512],
                                        start=(kc == 0),
                                        stop=(kc == NKC - 1),
                                    )
                        for hh in range(2):
                            rt = small.tile([1, 512], f32, tag="rt", name=f"rt_{pr}_{qi}_{hh}")
                            nc.vector.reciprocal(rt[0:1, :], oaccs[hh][64:65, :])
                            rb = small.tile([64, 512], f32, tag="rb", name=f"rb_{pr}_{qi}_{hh}")
                            nc.gpsimd.partition_broadcast(rb[:, :], rt[0:1, :], channels=64)
                            nc.vector.tensor_tensor(
                                out=ots[pr][hh * 64 : hh * 64 + 64, qs : qs + 512],
                                in0=oaccs[hh][0:64, :],
                                in1=rb[:, :],
                                op=MUL,
                            )
                    if phase == "attn":
                        continue
                    # wo for the 4 t-chunks of this q-tile; Y packed into
                    # [128, 2048] tiles = 2 t-chunks -> 2 output DMAs per qtile
                    for yt in range(2):
                        ysb = ypool.tile([128, 2048], f32, tag="y", name=f"y_{qi}_{yt}")
                        for sub in range(2):
                            tt = 4 * qi + 2 * yt + sub
                            for eh in range(2):
                                yps = psb.tile(
                                    [128, 512], f32, tag="b1", name=f"yp_{tt}_{eh}"
                                )
                                for r in range(2):
                                    nc.tensor.matmul(
                                        yps[:],
                                        ots[r][:, tt * 128 : (tt + 1) * 128],
                                        wo_t[:, r * 1024 + eh * 512 : r * 1024 + (eh + 1) * 512],
                                        start=(r == 0),
                                        stop=(r == 1),
                                    )
                                nc.vector.tensor_copy(
                                    ysb[:, sub * 1024 + eh * 512 : sub * 1024 + (eh + 1) * 512],
                                    yps[:],
                                )
                        eng = nc.sync if (qi * 2 + yt) % 2 == 0 else nc.scalar
                        eng.dma_start(
                            out=y_r[:, 4 * qi + 2 * yt : 4 * qi + 2 * yt + 2, :],
                            in_=ysb.rearrange("p (b e) -> p b e", e=1024),
                        )

                if phase == "attn":
                    dbg2 = ypool.tile([128, 2048], f32, tag="y", name="dbg2")
                    nc.vector.tensor_copy(dbg2[:, 0:256], ots[0].bitcast(f32)[:, 0:256])
                    nc.sync.dma_start(
                        out=y_r[:, 0:2, :], in_=dbg2.rearrange("p (b e) -> p b e", e=1024)
                    )

            if iters:
                import concourse.mybir as _mb
                with tc.For_i(
                    0,
                    iters,
                    1,
                    hint_engines=(
                        _mb.EngineType.PE,
                        _mb.EngineType.Activation,
                        _mb.EngineType.DVE,
                        _mb.EngineType.SP,
                        _mb.EngineType.Pool,
                    ),
                    staggered_reset=True,
                ) as _iv:
                    body()
            else:
                body()

    nc.compile()
    return nc


def _host_inputs(x, wq_w, wq_b, wk_w, wk_b, wv_w, wv_b, wo_w, wo_b):
    """Build the 8 per-core input maps (all host-side slicing/packing)."""
    f = np.float32
    x = np.asarray(x, f)
    wq_w = np.asarray(wq_w, f)
    wk_w = np.asarray(wk_w, f)
    wv_w = np.asarray(wv_w, f)
    wo_w = np.asarray(wo_w, f)
    wq_b = np.asarray(wq_b, f)
    wk_b = np.asarray(wk_b, f)
    wv_b = np.asarray(wv_b, f)
    wo_b = np.asarray(wo_b, f)

    def chunkpack(a, ncol):  # [1024, ncol] -> [128, 8*ncol] (D-chunk packed)
        return np.ascontiguousarray(
            a.reshape(NDC, 128, ncol).transpose(1, 0, 2).reshape(128, NDC * ncol)
        )

    # RoPE tables in fp32, mirroring the reference formulas.
    pos = np.arange(T, dtype=f)[:, None]
    idx = np.arange(HALF, dtype=f)[None, :]
    inv_freq = (f(1.0) / (f(ROPE_BASE) ** (idx / f(HALF)))).astype(f)
    ang = pos * inv_freq  # [T, 32]
    cosv, sinv = np.cos(ang).astype(f), np.sin(ang).astype(f)
    cos64 = np.repeat(cosv.T, 2, axis=0)  # [64, T]
    sin64 = np.repeat(sinv.T, 2, axis=0)
    sin64[0::2] *= -1  # rows 2j: -sin, rows 2j+1: +sin
    cos128 = np.tile(cos64, (2, 1))
    sin128 = np.tile(sin64, (2, 1))

    perm64 = np.empty(64, np.int64)
    perm64[0::2] = np.arange(32)
    perm64[1::2] = np.arange(32) + 32

    xtp = [
        np.ascontiguousarray(
            x[b].T.reshape(NDC, 128, T).transpose(1, 0, 2).reshape(128, NDC * T)
        )
        for b in range(B)
    ]

    in_maps = []
    for c in range(N_CORES):
        b, g = c // 4, c % 4
        heads = np.arange(4 * g, 4 * g + 4)
        qk_cols = np.concatenate([h * 64 + perm64 for h in heads])
        v_cols = np.concatenate([np.arange(h * 64, (h + 1) * 64) for h in heads])
        wp1 = np.concatenate(
            [chunkpack(wq_w[:, qk_cols], 256), chunkpack(wk_w[:, qk_cols], 256)],
            axis=1,
        )
        wp2 = np.concatenate(
            [
                chunkpack(wv_w[:, v_cols], 256),
                cos128,
                sin128,
                wq_b[qk_cols].reshape(2, 128).T,
                wk_b[qk_cols].reshape(2, 128).T,
            ],
            axis=1,
        )
        wop = np.ascontiguousarray(
            wo_w[v_cols, :]
            .reshape(2, 128, D)
            .transpose(1, 0, 2)
            .reshape(128, 2 * D)
            .astype(BF16)
        )
        in_maps.append(
            {
                "xtp": xtp[b],
                "wp1": np.ascontiguousarray(wp1),
                "wp2": np.ascontiguousarray(wp2),
                "wop": wop,
            }
        )

    beff = (wo_b.astype(np.float64) + wv_b.astype(np.float64) @ wo_w.astype(np.float64)).astype(f)
    return in_maps, beff


def kernel(x, wq_w, wq_b, wk_w, wk_b, wv_w, wv_b, wo_w, wo_b):
    from concourse import bass2jax

    in_maps, beff = _host_inputs(
        x, wq_w, wq_b, wk_w, wk_b, wv_w, wv_b, wo_w, wo_b
    )
    if "nc" not in _ctx:
        _ctx["nc"] = _build_nc(0)
    res = bass2jax.run_bass_via_pjrt(_ctx["nc"], in_maps, n_cores=N_CORES)
    y = np.empty((B, T, D), np.float32)
    for b in range(B):
        acc = res[4 * b]["y"].copy()
        for g in range(1, 4):
            acc += res[4 * b + g]["y"]
        y[b] = acc + beff[None, :]
    return y
